# revision 5
# baseline (speedup 1.0000x reference)
"""Int4 grouped-quantized Linear (GPTQ-style) on 8 Trainium2 NeuronCores.

y = x @ W + bias, W[i,o] = q[i,o] * scales[i//128, o] - zeros[i//128, o],
q packed 8 nibbles per int32 along in_features.

Strategy (column-parallel, per sharding hint):
  - shard q_weights/scales/zeros/bias along out_features across 8 cores
    (512 out columns per core); replicate x.
  - host: unpack nibbles to uint8 (pure layout transform), cast x to bf16
    and pre-tile it as [ssc, it, 128, 512] so every DMA is contiguous.
  - device: dequantize W slice to bf16 once (DVE: (q * s_rep) - z_rep with
    scales/zeros partition-broadcast by DMA), then stream x through the
    TensorEngine: out[sc 128 x 512] accumulated over 32 k-tiles in PSUM,
    bias added on the PSUM->SBUF move, DMA to HBM.
  - host: concat the 8 [8192, 512] slices along out_features.
"""

import numpy as np
import ml_dtypes

BF16 = ml_dtypes.bfloat16

B, S, IN_F, OUT_F = 4, 2048, 4096, 4096
BS = B * S                    # 8192 flattened rows
PACK = 8                      # nibbles per int32
N_CORES = 8
O_LOC = OUT_F // N_CORES      # 512 out columns per core
N_IT = IN_F // 128            # 32 contraction tiles
F_CHUNK = 512                 # x columns staged per buffer
SUB_PER = F_CHUNK // 128      # 4 matmul groups per staged chunk
N_SSC = BS // F_CHUNK         # 16


def _build_program(n_ssc=N_SSC):
    import concourse.bass as bass  # noqa: F401
    import concourse.tile as tile
    from concourse import bacc, mybir

    Alu = mybir.AluOpType  # noqa: F841
    dt = mybir.dt
    bs = n_ssc * F_CHUNK

    # Bacc (not bare Bass): its compile() pipeline runs
    # generate_event_semaphores, which splits instructions with >1 sem wait
    # into hardware-legal form — walrus rejects multi-wait instructions.
    nc = bacc.Bacc(None)
    xt4 = nc.declare_dram_parameter(
        "xt4", [n_ssc, N_IT, 128, F_CHUNK], dt.bfloat16, False)
    qu8 = nc.declare_dram_parameter("qu8", [N_IT, 128, O_LOC], dt.uint8, False)
    sca = nc.declare_dram_parameter("sca", [N_IT, O_LOC], dt.float32, False)
    zer = nc.declare_dram_parameter("zer", [N_IT, O_LOC], dt.float32, False)
    brep = nc.declare_dram_parameter("brep", [128, O_LOC], dt.float32, False)
    y = nc.declare_dram_parameter("y", [bs, O_LOC], dt.float32, True)

    with tile.TileContext(nc) as tc:
        with (
            tc.tile_pool(name="wpool", bufs=1) as wpool,
            tc.tile_pool(name="dq", bufs=2) as dq,
            tc.tile_pool(name="xin", bufs=2) as xin,
            tc.tile_pool(name="pp", bufs=6, space="PSUM") as pp,
            tc.tile_pool(name="op", bufs=4) as op_pool,
            tc.tile_pool(name="cst", bufs=1) as cst,
        ):
            bias_sb = cst.tile([128, O_LOC], dt.float32, tag="bias")
            nc.sync.dma_start(bias_sb[:], brep[:])

            # ---- dequantize the per-core W slice into 32 bf16 k-tiles ----
            w_tiles = []
            for it in range(N_IT):
                qt = dq.tile([128, O_LOC], dt.uint8, tag="qt")
                nc.sync.dma_start(qt[:], qu8[it])
                srep = dq.tile([128, O_LOC], dt.float32, tag="srep")
                nc.sync.dma_start(
                    srep[:], sca[it : it + 1, :].broadcast_to([128, O_LOC]))
                zrep = dq.tile([128, O_LOC], dt.float32, tag="zrep")
                nc.sync.dma_start(
                    zrep[:], zer[it : it + 1, :].broadcast_to([128, O_LOC]))
                qs = dq.tile([128, O_LOC], dt.float32, tag="qs")
                nc.vector.tensor_mul(qs[:], qt[:], srep[:])
                wt = wpool.tile([128, O_LOC], dt.bfloat16, tag=f"w{it}")
                nc.vector.tensor_sub(wt[:], qs[:], zrep[:])
                w_tiles.append(wt)

            # ---- main matmul stream ----
            for ssc in range(n_ssc):
                xts = []
                for it in range(N_IT):
                    xt_ = xin.tile([128, F_CHUNK], dt.bfloat16, tag=f"x{it}")
                    nc.sync.dma_start(xt_[:], xt4[ssc, it])
                    xts.append(xt_)
                for sub in range(SUB_PER):
                    sc = ssc * SUB_PER + sub
                    ps = pp.tile([128, O_LOC], dt.float32, tag="ps")
                    for it in range(N_IT):
                        nc.tensor.matmul(
                            ps[:],
                            xts[it][:, sub * 128 : (sub + 1) * 128],
                            w_tiles[it][:],
                            start=(it == 0),
                            stop=(it == N_IT - 1),
                        )
                    ot = op_pool.tile([128, O_LOC], dt.float32, tag="ot")
                    nc.vector.tensor_add(ot[:], ps[:], bias_sb[:])
                    nc.sync.dma_start(y[sc * 128 : (sc + 1) * 128, :], ot[:])
    return nc


def _prep_shared(x, q_weights, n_ssc=N_SSC):
    bs = n_ssc * F_CHUNK
    x2 = x.reshape(-1, IN_F)[:bs]
    xb = np.ascontiguousarray(x2).astype(BF16)
    # xt4[ssc, it, r, f] = x[ssc*F_CHUNK + f, it*128 + r]
    xt4 = np.ascontiguousarray(
        xb.reshape(n_ssc, F_CHUNK, N_IT, 128).transpose(0, 2, 3, 1))
    shifts = np.arange(PACK, dtype=np.int32) * 4
    nib = (q_weights[:, None, :] >> shifts[None, :, None]) & np.int32(0xF)
    q_all = nib.astype(np.uint8).reshape(IN_F, OUT_F)
    return xt4, q_all


def _core_inputs(xt4, q_all, scales, zeros, bias, c):
    sl = slice(c * O_LOC, (c + 1) * O_LOC)
    return {
        "xt4": xt4,
        "qu8": np.ascontiguousarray(q_all[:, sl]).reshape(N_IT, 128, O_LOC),
        "sca": np.ascontiguousarray(scales[:, sl]),
        "zer": np.ascontiguousarray(zeros[:, sl]),
        "brep": np.ascontiguousarray(
            np.broadcast_to(bias[sl][None, :], (128, O_LOC))),
    }


def _ensure_axon_trace_hook():
    """Some images lack antenv.axon_hooks; bass_utils imports it whenever
    tracing is requested (trace=True or BASS_TRACE=1). Recreate it from
    trn_agent_boot so tracing works instead of crashing; degrade silently
    if the boot machinery isn't available either."""
    import sys as _sys
    import types as _types
    try:
        import antenv.axon_hooks  # noqa: F401
        return
    except ImportError:
        pass
    try:
        import antenv
        from trn_agent_boot.trn_boot import _ntff_profile_via_ctypes

        hook = _ntff_profile_via_ctypes("/opt/axon/libaxon_pjrt.so")
        mod = _types.ModuleType("antenv.axon_hooks")
        mod.get_axon_ntff_profile_hook = lambda: hook
        mod.set_axon_ntff_profile_hook = lambda h: None
        _sys.modules["antenv.axon_hooks"] = mod
        antenv.axon_hooks = mod
    except Exception:
        pass


def _run(x, q_weights, scales, zeros, bias, trace=False, **kwargs):
    _ensure_axon_trace_hook()
    from concourse.bass_utils import run_bass_kernel_spmd

    nc = _build_program()
    if not nc.is_finalized():
        nc.finalize()  # runs Bacc.compile(): reg alloc + event-sem legalization
    xt4, q_all = _prep_shared(x, q_weights)
    in_maps = [
        _core_inputs(xt4, q_all, scales, zeros, bias, c) for c in range(N_CORES)
    ]
    res = run_bass_kernel_spmd(
        nc, in_maps, list(range(N_CORES)), trace=trace, **kwargs)
    y = np.concatenate([res.results[c]["y"] for c in range(N_CORES)], axis=1)
    return np.ascontiguousarray(y.reshape(B, S, OUT_F), dtype=np.float32), res


def kernel(x, q_weights, scales, zeros, bias):
    x = np.asarray(x, dtype=np.float32)
    q_weights = np.asarray(q_weights, dtype=np.int32)
    scales = np.asarray(scales, dtype=np.float32)
    zeros = np.asarray(zeros, dtype=np.float32)
    bias = np.asarray(bias, dtype=np.float32)
    y, _ = _run(x, q_weights, scales, zeros, bias)
    return y
